# revision 21
# baseline (speedup 1.0000x reference)
"""Trainium2 Bass kernel for nn_BoostEnhancedAttention.

Reference computation:
    v   = (values @ W_v.T + b_v)                      # [B, NK, H*D_V]
    att = softmax(att3 * att12 interleaved, axis=k)   # [B, H, NQ, NK]
    out = (att @ v_per_head) @ W_o.T + b_o            # [B, NQ, D_MODEL]

Restructuring (exact algebra, validated vs reference):
  - Scores factor as s[b,h,q,k] = att3[b,h,q,c(k)] * att12[b,h,...f(k)];
    exp(s) computed by ACT from a DVE-built product grid.
  - Softmax-linearity fold: per-head M_h = W_o[:,h] @ W_v[h,:] applied
    AFTER attention: out[b] = sum_h (att_h @ values[b]) @ M_h.T + b_eff.
  - Shifted-softmax fp8 split: E = 1 + W with W = exp(s) - 1.  Then
        G = sum_k E_k v_k = Vsum + sum_k W_k v_k,   Z = NK + sum_k W_k
    where Vsum = sum_k values[k,:] is computed EXACTLY on the host (f32)
    and injected through a K=1 matmul.  Only the small residual W (rms
    ~0.29x of E) flows through fp8, so both W and values quantize to
    fp8-e4m3 within the error budget.  This enables DoubleRow matmuls:
    each PE instruction contracts TWO 128-key tiles (K=256) at 0.5
    cycles/row - halving both the matmul count and the PE-bound time.
  - Z accumulated on the PE with fp8 DoubleRow ones-matmuls (output
    replicated across partitions), removing the serial DVE add chain.

Sharding: data-parallel over batch, B=32 over 8 cores -> 4 batches/core.
No collectives; outputs concatenated on host.
"""

import numpy as np
import ml_dtypes

B, CH, CW, H, FH, FW = 32, 16, 16, 8, 4, 4
NQ = 64
NCELL = CH * CW          # 256 coarse cells (c)
F = FH * FW              # 16 fine positions per cell
NK = NCELL * F           # 4096
D_IN, D_V, D_MODEL = 512, 64, 512
N_CORES = 8
B_LOC = B // N_CORES     # 4
N_KP = 16                # k-tile PAIRS of 256 keys: kp = half*8 + f//2
N_DT = 4                 # d_in tiles of 128
HQ = H * NQ              # 512

BF16 = ml_dtypes.bfloat16
FP8 = ml_dtypes.float8_e4m3


def _k_perm():
    """perm[k'] -> original k, where k' = (half*16+f)*128 + c_loc.

    Original key order is (ch, fh, cw, fw):  k = ch*256 + fh*64 + cw*4 + fw.
    New order groups a k-tile as (fixed f=(fh,fw), c = half*128 + c_loc).
    """
    perm = np.zeros(NK, np.int64)
    c = np.arange(NCELL)
    ch_i, cw_i = c // CW, c % CW
    for half in range(2):
        for f in range(F):
            kt = half * F + f
            fh, fw = f // FW, f % FW
            cc = half * 128 + np.arange(128)
            perm[kt * 128:(kt + 1) * 128] = (
                ch_i[cc] * (FH * CW * FW) + fh * (CW * FW) + cw_i[cc] * FW + fw
            )
    return perm


_PERM = _k_perm()
_NC_CACHE = {}


def _build_nc():
    from contextlib import ExitStack

    import concourse.bass as bass
    import concourse.tile as tile
    from concourse import bacc, mybir

    f32 = mybir.dt.float32
    bf16 = mybir.dt.bfloat16
    f8 = mybir.dt.float8e4
    DR = mybir.MatmulPerfMode.DoubleRow

    nc = bacc.Bacc("TRN2", target_bir_lowering=False, debug=False,
                   num_devices=N_CORES)

    # values pre-paired on host: [b, kp, p, i*512+d] = values[b, (2kp+i)*128+p, d]
    values_p = nc.dram_tensor("values_p", [B_LOC, N_KP, 128, 2 * D_IN], f8,
                              kind="ExternalInput")
    vsum = nc.dram_tensor("vsum", [B_LOC, D_IN], bf16, kind="ExternalInput")
    att3_t = nc.dram_tensor("att3_t", [B_LOC, NCELL, HQ], bf16,
                            kind="ExternalInput")
    att12_pair = nc.dram_tensor("att12_pair", [B_LOC, NCELL, F * H * 2], bf16,
                                kind="ExternalInput")
    m_all = nc.dram_tensor("m_all", [128, N_DT * H * D_MODEL], bf16,
                           kind="ExternalInput")
    out = nc.dram_tensor("out", [B_LOC * NQ, D_MODEL], f32,
                         kind="ExternalOutput")

    with tile.TileContext(nc) as tc, ExitStack() as ctx:
        const_pool = ctx.enter_context(tc.tile_pool(name="const", bufs=1))
        a3_pool = ctx.enter_context(tc.tile_pool(name="a3", bufs=2))
        a12r_pool = ctx.enter_context(tc.tile_pool(name="a12r", bufs=2))
        vs_pool = ctx.enter_context(tc.tile_pool(name="vs", bufs=2))
        vt_pool = ctx.enter_context(tc.tile_pool(name="vt", bufs=20))
        sc_pool = ctx.enter_context(tc.tile_pool(name="sc", bufs=4))
        et_pool = ctx.enter_context(tc.tile_pool(name="et", bufs=4))
        w8_pool = ctx.enter_context(tc.tile_pool(name="w8", bufs=8))
        zb_pool = ctx.enter_context(tc.tile_pool(name="zb", bufs=2))
        g_pool = ctx.enter_context(tc.tile_pool(name="gps", bufs=1, space="PSUM"))
        z_pool = ctx.enter_context(tc.tile_pool(name="zps", bufs=1, space="PSUM"))
        o_pool = ctx.enter_context(tc.tile_pool(name="ops", bufs=1, space="PSUM"))
        o_sb_pool = ctx.enter_context(tc.tile_pool(name="osb", bufs=2))

        ones_sb = const_pool.tile([128, 128], bf16)
        nc.vector.memset(ones_sb[:], 1.0)
        # fp8 ones for the Z DoubleRow matmuls: [128, 2, 128] -> free 256
        ones8 = const_pool.tile([128, 256], f8)
        nc.vector.memset(ones8[:], 1.0)
        # K=1 helpers: streaming row of ones, and the NK offset for Z
        ones_row = const_pool.tile([1, HQ], bf16)
        nc.vector.memset(ones_row[:], 1.0)
        neg1 = const_pool.tile([128, 1], f32)
        nc.vector.memset(neg1[:], -1.0)
        zinit = const_pool.tile([1, 128], bf16)
        nc.vector.memset(zinit[:], float(NK))
        warm_sb = const_pool.tile([128, D_MODEL], bf16, name="warm_sb")
        nc.vector.memset(warm_sb[:], 1.0)
        warm = o_pool.tile([128, D_MODEL], f32, tag="o", name="warm")
        for wi in range(12):
            nc.tensor.matmul(warm[:], ones_sb[:], warm_sb[:],
                             start=True, stop=True)
        # g_all[d_loc, (dt, h, b, q)] : normalized attention output, bf16
        g_all = const_pool.tile([128, N_DT * H * B_LOC * NQ], bf16)

        Q2 = NQ // 2

        def emit_group(b, half, gi, FQ, f0, a3_t, a12r_t):
            """One score group: broadcast multiply, exp, and W=E-1 in fp8."""
            a3b = a3_t[half][:]
            in0 = bass.AP(a3b.tensor, a3b.offset,
                          [a3b.ap[0], [0, FQ], [NQ, H], [2, Q2], [1, 2]])
            sc = sc_pool.tile([128, 8 * HQ], bf16, tag="sc",
                              name=f"sc_{b}_{half}_{gi}")
            scb = sc[:]
            out_ap = bass.AP(scb.tensor, scb.offset,
                             [scb.ap[0], [HQ, FQ], [NQ, H], [2, Q2], [1, 2]])
            a12b = a12r_t[half][:]
            in1 = bass.AP(a12b.tensor, a12b.offset + f0 * H * 2,
                          [a12b.ap[0], [H * 2, FQ], [2, H], [0, Q2], [1, 2]])
            nc.vector.tensor_mul(out_ap, in0, in1)
            et = et_pool.tile([128, 8 * HQ], bf16, tag="et",
                              name=f"et_{b}_{half}_{gi}")
            nc.scalar.activation(et[:, :FQ * HQ], sc[:, :FQ * HQ],
                                 mybir.ActivationFunctionType.Exp)
            w8 = w8_pool.tile([128, 8 * HQ], f8, tag="w8",
                              name=f"w8_{b}_{half}_{gi}")
            # W = E - 1 (GpSimd measured 23x slower here: fp8 convert is a
            # software path in the Q7 ucode — keep on DVE/ACT).  The last 512
            # columns go to the Scalar engine to balance per-group rates:
            # DVE mul+wsub was ~4.6us/group vs PE 4.3us.
            cut = FQ * HQ - HQ
            nc.vector.tensor_scalar_sub(w8[:, :cut], et[:, :cut], 1.0)
            nc.scalar.activation(w8[:, cut:FQ * HQ], et[:, cut:FQ * HQ],
                                 mybir.ActivationFunctionType.Identity,
                                 bias=neg1[:])
            return w8

        def prologue(b):
            """Input DMAs + first score group for batch b."""
            # half-0 inputs first so group 0's multiply can start after 2 DMAs
            a3_t = [a3_pool.tile([128, HQ], bf16, tag=f"a3_{hf}",
                                 name=f"a3_{b}_{hf}") for hf in range(2)]
            a12r_t = [a12r_pool.tile([128, F * H * 2], bf16, tag=f"a12r_{hf}",
                                     name=f"a12r_{b}_{hf}") for hf in range(2)]
            for hf in range(2):
                nc.sync.dma_start(a3_t[hf][:],
                                  att3_t.ap()[b, hf * 128:(hf + 1) * 128, :])
                nc.sync.dma_start(a12r_t[hf][:],
                                  att12_pair.ap()[b, hf * 128:(hf + 1) * 128, :])
            vs = vs_pool.tile([1, D_IN], bf16, tag="vs", name=f"vs_{b}")
            nc.sync.dma_start(vs[:], vsum.ap()[b:b + 1, :])
            # emit ALL of half 0's score groups here so their DVE/ACT work
            # precedes the previous batch's recip+norm in the engine queues —
            # otherwise the PE starves for W8 at every batch transition
            groups = [2, 2, 4, 4, 4] if b == 0 else [8, 8]
            w_h0 = []
            f0 = 0
            for gi, FQ in enumerate(groups):
                w_h0.append(emit_group(b, 0, gi, FQ, f0, a3_t, a12r_t))
                f0 += FQ
            return a3_t, a12r_t, vs, groups, w_h0

        # m_sb DMA is split into 8 chunks spread over early half-windows so it
        # never stalls the critical vt2 prefetch stream (needed first at b=1).
        m_sb = const_pool.tile([128, N_DT * H * D_MODEL], bf16, name="m_sb")
        mq = N_DT * H * D_MODEL // 8

        DEFER_P = 6  # pairs whose dt>=1 matmuls lag, easing PSUM bank reuse
                     # and deepening the W8 pipeline before PE ramps up

        pro = prologue(0)
        for b in range(B_LOC):
            a3_t, a12r_t, vs, groups0, w_h0 = pro
            gps = [g_pool.tile([128, HQ], f32, tag=f"g{dt}", name=f"g_{b}_{dt}",
                               bufs=(2 if dt == 0 else 1))
                   for dt in range(N_DT)]
            zps = z_pool.tile([128, HQ], f32, tag="z", name=f"z_{b}", bufs=2)

            # Accumulation openers: Vsum (exact, host) and the NK shift of Z,
            # injected by K=1 matmuls that broadcast a row across PSUM.
            # dt>=1 openers are deferred with their matmuls.
            vsb = vs[:]
            nc.tensor.matmul(gps[0][:], vsb[:, 0:128], ones_row[:],
                             start=True, stop=False)
            nc.tensor.matmul(zps[:], zinit[:], ones_row[:],
                             start=True, stop=False)

            deferred = []
            for half in range(2):
                groups = groups0 if half == 0 else [8, 8]
                f0 = 0
                for gi, FQ in enumerate(groups):
                    if half == 0:
                        w8 = w_h0[gi]
                    else:
                        w8 = emit_group(b, half, gi, FQ, f0, a3_t, a12r_t)
                    w8b = w8[:]
                    for j2 in range(FQ // 2):
                        kp = half * 8 + (f0 + j2 * 2) // 2
                        last = kp == N_KP - 1
                        vt2 = vt_pool.tile([128, 2 * D_IN], f8, tag="vt",
                                           name=f"vt_{b}_{kp}")
                        nc.sync.dma_start(vt2[:], values_p.ap()[b, kp])
                        vtb = vt2[:]
                        rhs = bass.AP(w8b.tensor, w8b.offset + (j2 * 2) * HQ,
                                      [w8b.ap[0], [HQ, 2], [1, HQ]])

                        def g_mm(dt, vtb=vtb, rhs=rhs, last=last):
                            lhsT = bass.AP(vtb.tensor, vtb.offset + dt * 128,
                                           [vtb.ap[0], [D_IN, 2], [1, 128]])
                            nc.tensor.matmul(gps[dt][:], lhsT, rhs,
                                             start=False, stop=last,
                                             perf_mode=DR)

                        g_mm(0)
                        o8 = ones8[:]
                        z_lhsT = bass.AP(o8.tensor, o8.offset,
                                         [o8.ap[0], [128, 2], [1, 128]])
                        nc.tensor.matmul(zps[:], z_lhsT, rhs,
                                         start=False, stop=last, perf_mode=DR)
                        if kp < DEFER_P:
                            deferred.append(g_mm)
                            if kp == DEFER_P - 1:
                                for dt in range(1, N_DT):
                                    nc.tensor.matmul(
                                        gps[dt][:],
                                        vsb[:, dt * 128:(dt + 1) * 128],
                                        ones_row[:], start=True, stop=False)
                                    for mm in deferred:
                                        mm(dt)
                        else:
                            for dt in range(1, N_DT):
                                g_mm(dt)
                    f0 += FQ
                win = b * 2 + half
                if win < 4:  # spread m_sb: 2 chunks per half-window of b0/b1
                    for mi in (2 * win, 2 * win + 1):
                        nc.sync.dma_start(m_sb[:, mi * mq:(mi + 1) * mq],
                                          m_all.ap()[:, mi * mq:(mi + 1) * mq])

            if b + 1 < B_LOC:
                pro = prologue(b + 1)

            zb = zb_pool.tile([128, HQ], f32)
            nc.vector.reciprocal_approx_fast(zb[:], zps[:])

            ga_v = g_all[:].rearrange("p (dt h bb q) -> p dt h bb q",
                                      dt=N_DT, h=H, bb=B_LOC)
            if b % 2 == 1:
                # interleave normalization with the output projection so the
                # last batch's tail is short: norm(dt) then its 8 proj matmuls
                bq = b // 2
                ops = o_pool.tile([128, D_MODEL], f32, tag="o")
                for dt in range(N_DT):
                    nc.vector.tensor_mul(
                        ga_v[:, dt, :, b, :],
                        gps[dt][:].rearrange("p (h q) -> p h q", h=H),
                        zb[:].rearrange("p (h q) -> p h q", h=H),
                    )
                    for h in range(H):
                        col = dt * (H * B_LOC * NQ) + h * (B_LOC * NQ) + bq * 128
                        nc.tensor.matmul(
                            ops[:],
                            g_all[:, col:col + 128],
                            m_sb[:, (dt * H + h) * D_MODEL:
                                 (dt * H + h + 1) * D_MODEL],
                            start=(dt == 0 and h == 0),
                            stop=(dt == N_DT - 1 and h == H - 1))
                # b_eff is added on the host
                out_sb = o_sb_pool.tile([128, D_MODEL], f32, tag="osb",
                                        name=f"osb_{bq}")
                nc.vector.tensor_copy(out_sb[:], ops[:])
                nc.sync.dma_start(out.ap()[bq * 128:(bq + 1) * 128, :],
                                  out_sb[:])
            else:
                for dt in range(N_DT):
                    nc.vector.tensor_mul(
                        ga_v[:, dt, :, b, :],
                        gps[dt][:].rearrange("p (h q) -> p h q", h=H),
                        zb[:].rearrange("p (h q) -> p h q", h=H),
                    )

    nc.compile()
    return nc


def _get_nc():
    if "nc" not in _NC_CACHE:
        _NC_CACHE["nc"] = _build_nc()
    return _NC_CACHE["nc"]


def _host_prep(att12, att3, values, W_v, b_v, W_o, b_o):
    att12 = np.asarray(att12, np.float32)
    att3 = np.asarray(att3, np.float32)
    values = np.asarray(values, np.float32)
    W_v = np.asarray(W_v, np.float32)
    b_v = np.asarray(b_v, np.float32)
    W_o = np.asarray(W_o, np.float32)
    b_o = np.asarray(b_o, np.float32)

    # paired fp8 values: [b, kp, p, i*512+d] = values[b, perm[(2kp+i)*128+p], d]
    values_p = np.ascontiguousarray(
        values[:, _PERM, :].reshape(B, N_KP, 2, 128, D_IN)
        .transpose(0, 1, 3, 2, 4).reshape(B, N_KP, 128, 2 * D_IN)).astype(FP8)
    vsum = values.sum(axis=1).astype(BF16)              # [B, D_IN] exact f32 sum
    att3_t = np.ascontiguousarray(
        att3.transpose(0, 3, 1, 2).reshape(B, NCELL, HQ)).astype(BF16)
    att12_r = np.ascontiguousarray(
        att12.transpose(0, 1, 2, 4, 5, 3).reshape(B, NCELL, F * H)).astype(BF16)
    att12_pair = np.ascontiguousarray(np.broadcast_to(
        att12_r[:, :, :, None], (B, NCELL, F * H, 2)).reshape(
        B, NCELL, F * H * 2))

    # Per-head folded projection M_h = W_o_h @ W_v_h  [D_MODEL, D_IN]
    Wv3 = W_v.reshape(H, D_V, D_IN)
    Wo3 = W_o.reshape(D_MODEL, H, D_V)
    M = np.einsum("dhv,hvi->hdi", Wo3, Wv3)          # [H, DM, DIN]
    Mt = M.transpose(0, 2, 1)                        # [H, DIN, DM]
    m_all = np.ascontiguousarray(
        Mt.reshape(H, N_DT, 128, D_MODEL).transpose(2, 1, 0, 3)
        .reshape(128, N_DT * H * D_MODEL)).astype(BF16)

    b_eff = (b_o + np.einsum("dhv,hv->d", Wo3,
                             b_v.reshape(H, D_V))).astype(np.float32)
    return values_p, vsum, att3_t, att12_pair, m_all, b_eff


def kernel(att12, att3, values, W_v, b_v, W_o, b_o):
    from concourse.bass_utils import run_bass_kernel_spmd

    values_p, vsum, att3_t, att12_pair, m_all, b_eff = _host_prep(
        att12, att3, values, W_v, b_v, W_o, b_o)

    in_maps = []
    for core in range(N_CORES):
        s = slice(core * B_LOC, (core + 1) * B_LOC)
        in_maps.append({
            "values_p": np.ascontiguousarray(values_p[s]),
            "vsum": np.ascontiguousarray(vsum[s]),
            "att3_t": np.ascontiguousarray(att3_t[s]),
            "att12_pair": np.ascontiguousarray(att12_pair[s]),
            "m_all": m_all,
        })

    nc = _get_nc()
    res = run_bass_kernel_spmd(nc, in_maps, core_ids=list(range(N_CORES)))
    out = np.concatenate(
        [res.results[i]["out"].reshape(B_LOC, NQ, D_MODEL)
         for i in range(N_CORES)], axis=0)
    return out.astype(np.float32) + b_eff


# revision 22
# speedup vs baseline: 1.0073x; 1.0073x over previous
"""Trainium2 Bass kernel for nn_BoostEnhancedAttention.

Reference computation:
    v   = (values @ W_v.T + b_v)                      # [B, NK, H*D_V]
    att = softmax(att3 * att12 interleaved, axis=k)   # [B, H, NQ, NK]
    out = (att @ v_per_head) @ W_o.T + b_o            # [B, NQ, D_MODEL]

Restructuring (exact algebra, validated vs reference):
  - Scores factor as s[b,h,q,k] = att3[b,h,q,c(k)] * att12[b,h,...f(k)];
    exp(s) computed by ACT from a DVE-built product grid.
  - Softmax-linearity fold: per-head M_h = W_o[:,h] @ W_v[h,:] applied
    AFTER attention: out[b] = sum_h (att_h @ values[b]) @ M_h.T + b_eff.
  - Shifted-softmax fp8 split: E = 1 + W with W = exp(s) - 1.  Then
        G = sum_k E_k v_k = Vsum + sum_k W_k v_k,   Z = NK + sum_k W_k
    where Vsum = sum_k values[k,:] is computed EXACTLY on the host (f32)
    and injected through a K=1 matmul.  Only the small residual W (rms
    ~0.29x of E) flows through fp8, so both W and values quantize to
    fp8-e4m3 within the error budget.  This enables DoubleRow matmuls:
    each PE instruction contracts TWO 128-key tiles (K=256) at 0.5
    cycles/row - halving both the matmul count and the PE-bound time.
  - Z accumulated on the PE with fp8 DoubleRow ones-matmuls (output
    replicated across partitions), removing the serial DVE add chain.

Sharding: data-parallel over batch, B=32 over 8 cores -> 4 batches/core.
No collectives; outputs concatenated on host.
"""

import numpy as np
import ml_dtypes

B, CH, CW, H, FH, FW = 32, 16, 16, 8, 4, 4
NQ = 64
NCELL = CH * CW          # 256 coarse cells (c)
F = FH * FW              # 16 fine positions per cell
NK = NCELL * F           # 4096
D_IN, D_V, D_MODEL = 512, 64, 512
N_CORES = 8
B_LOC = B // N_CORES     # 4
N_KP = 16                # k-tile PAIRS of 256 keys: kp = half*8 + f//2
N_DT = 4                 # d_in tiles of 128
HQ = H * NQ              # 512

BF16 = ml_dtypes.bfloat16
FP8 = ml_dtypes.float8_e4m3


def _k_perm():
    """perm[k'] -> original k, where k' = (half*16+f)*128 + c_loc.

    Original key order is (ch, fh, cw, fw):  k = ch*256 + fh*64 + cw*4 + fw.
    New order groups a k-tile as (fixed f=(fh,fw), c = half*128 + c_loc).
    """
    perm = np.zeros(NK, np.int64)
    c = np.arange(NCELL)
    ch_i, cw_i = c // CW, c % CW
    for half in range(2):
        for f in range(F):
            kt = half * F + f
            fh, fw = f // FW, f % FW
            cc = half * 128 + np.arange(128)
            perm[kt * 128:(kt + 1) * 128] = (
                ch_i[cc] * (FH * CW * FW) + fh * (CW * FW) + cw_i[cc] * FW + fw
            )
    return perm


_PERM = _k_perm()
_NC_CACHE = {}


def _build_nc():
    from contextlib import ExitStack

    import concourse.bass as bass
    import concourse.tile as tile
    from concourse import bacc, mybir

    f32 = mybir.dt.float32
    bf16 = mybir.dt.bfloat16
    f8 = mybir.dt.float8e4
    DR = mybir.MatmulPerfMode.DoubleRow

    nc = bacc.Bacc("TRN2", target_bir_lowering=False, debug=False,
                   num_devices=N_CORES)

    # values pre-paired on host: [b, kp, p, i*512+d] = values[b, (2kp+i)*128+p, d]
    values_p = nc.dram_tensor("values_p", [B_LOC, N_KP, 128, 2 * D_IN], f8,
                              kind="ExternalInput")
    vsum = nc.dram_tensor("vsum", [B_LOC, D_IN], bf16, kind="ExternalInput")
    att3_t = nc.dram_tensor("att3_t", [B_LOC, NCELL, HQ], bf16,
                            kind="ExternalInput")
    att12_pair = nc.dram_tensor("att12_pair", [B_LOC, NCELL, F * H * 2], bf16,
                                kind="ExternalInput")
    m_all = nc.dram_tensor("m_all", [128, N_DT * H * D_MODEL], bf16,
                           kind="ExternalInput")
    out = nc.dram_tensor("out", [B_LOC * NQ, D_MODEL], f32,
                         kind="ExternalOutput")

    with tile.TileContext(nc) as tc, ExitStack() as ctx:
        const_pool = ctx.enter_context(tc.tile_pool(name="const", bufs=1))
        a3_pool = ctx.enter_context(tc.tile_pool(name="a3", bufs=2))
        a12r_pool = ctx.enter_context(tc.tile_pool(name="a12r", bufs=2))
        vs_pool = ctx.enter_context(tc.tile_pool(name="vs", bufs=2))
        vt_pool = ctx.enter_context(tc.tile_pool(name="vt", bufs=20))
        sc_pool = ctx.enter_context(tc.tile_pool(name="sc", bufs=4))
        et_pool = ctx.enter_context(tc.tile_pool(name="et", bufs=4))
        w8_pool = ctx.enter_context(tc.tile_pool(name="w8", bufs=8))
        zb_pool = ctx.enter_context(tc.tile_pool(name="zb", bufs=2))
        g_pool = ctx.enter_context(tc.tile_pool(name="gps", bufs=1, space="PSUM"))
        z_pool = ctx.enter_context(tc.tile_pool(name="zps", bufs=1, space="PSUM"))
        o_pool = ctx.enter_context(tc.tile_pool(name="ops", bufs=1, space="PSUM"))
        o_sb_pool = ctx.enter_context(tc.tile_pool(name="osb", bufs=2))

        ones_sb = const_pool.tile([128, 128], bf16)
        nc.vector.memset(ones_sb[:], 1.0)
        # fp8 ones for the Z DoubleRow matmuls: [128, 2, 128] -> free 256
        ones8 = const_pool.tile([128, 256], f8)
        nc.vector.memset(ones8[:], 1.0)
        # K=1 helpers: streaming row of ones, and the NK offset for Z
        ones_row = const_pool.tile([1, HQ], bf16)
        nc.vector.memset(ones_row[:], 1.0)
        neg1 = const_pool.tile([128, 1], f32)
        nc.vector.memset(neg1[:], -1.0)
        zinit = const_pool.tile([1, 128], bf16)
        nc.vector.memset(zinit[:], float(NK))
        warm_sb = const_pool.tile([128, D_MODEL], bf16, name="warm_sb")
        nc.vector.memset(warm_sb[:], 1.0)
        warm = o_pool.tile([128, D_MODEL], f32, tag="o", name="warm")
        for wi in range(12):
            nc.tensor.matmul(warm[:], ones_sb[:], warm_sb[:],
                             start=True, stop=True)
        # g_all[d_loc, (dt, h, b, q)] : normalized attention output, bf16
        g_all = const_pool.tile([128, N_DT * H * B_LOC * NQ], bf16)

        Q2 = NQ // 2

        def emit_group(b, half, gi, FQ, f0, a3_t, a12r_t):
            """One score group: broadcast multiply, exp, and W=E-1 in fp8."""
            a3b = a3_t[half][:]
            in0 = bass.AP(a3b.tensor, a3b.offset,
                          [a3b.ap[0], [0, FQ], [NQ, H], [2, Q2], [1, 2]])
            sc = sc_pool.tile([128, 8 * HQ], bf16, tag="sc",
                              name=f"sc_{b}_{half}_{gi}")
            scb = sc[:]
            out_ap = bass.AP(scb.tensor, scb.offset,
                             [scb.ap[0], [HQ, FQ], [NQ, H], [2, Q2], [1, 2]])
            a12b = a12r_t[half][:]
            in1 = bass.AP(a12b.tensor, a12b.offset + f0 * H * 2,
                          [a12b.ap[0], [H * 2, FQ], [2, H], [0, Q2], [1, 2]])
            nc.vector.tensor_mul(out_ap, in0, in1)
            et = et_pool.tile([128, 8 * HQ], bf16, tag="et",
                              name=f"et_{b}_{half}_{gi}")
            nc.scalar.activation(et[:, :FQ * HQ], sc[:, :FQ * HQ],
                                 mybir.ActivationFunctionType.Exp)
            w8 = w8_pool.tile([128, 8 * HQ], f8, tag="w8",
                              name=f"w8_{b}_{half}_{gi}")
            # W = E - 1 (GpSimd measured 23x slower here: fp8 convert is a
            # software path in the Q7 ucode — keep on DVE/ACT).  The last 512
            # columns go to the Scalar engine to balance per-group rates:
            # DVE mul+wsub was ~4.6us/group vs PE 4.3us.
            cut = FQ * HQ - HQ
            nc.vector.tensor_scalar_sub(w8[:, :cut], et[:, :cut], 1.0)
            nc.scalar.activation(w8[:, cut:FQ * HQ], et[:, cut:FQ * HQ],
                                 mybir.ActivationFunctionType.Identity,
                                 bias=neg1[:])
            return w8

        def prologue(b):
            """Input DMAs + first score group for batch b."""
            # half-0 inputs first so group 0's multiply can start after 2 DMAs
            a3_t = [a3_pool.tile([128, HQ], bf16, tag=f"a3_{hf}",
                                 name=f"a3_{b}_{hf}") for hf in range(2)]
            a12r_t = [a12r_pool.tile([128, F * H * 2], bf16, tag=f"a12r_{hf}",
                                     name=f"a12r_{b}_{hf}") for hf in range(2)]
            for hf in range(2):
                nc.sync.dma_start(a3_t[hf][:],
                                  att3_t.ap()[b, hf * 128:(hf + 1) * 128, :])
                nc.sync.dma_start(a12r_t[hf][:],
                                  att12_pair.ap()[b, hf * 128:(hf + 1) * 128, :])
            vs = vs_pool.tile([1, D_IN], bf16, tag="vs", name=f"vs_{b}")
            nc.sync.dma_start(vs[:], vsum.ap()[b:b + 1, :])
            # emit ALL of half 0's score groups here so their DVE/ACT work
            # precedes the previous batch's recip+norm in the engine queues —
            # otherwise the PE starves for W8 at every batch transition
            groups = [2, 2, 4, 4, 4] if b == 0 else [8, 8]
            w_h0 = []
            f0 = 0
            for gi, FQ in enumerate(groups):
                w_h0.append(emit_group(b, 0, gi, FQ, f0, a3_t, a12r_t))
                f0 += FQ
            return a3_t, a12r_t, vs, groups, w_h0

        # m_sb DMA is split into 8 chunks spread over early half-windows so it
        # never stalls the critical vt2 prefetch stream (needed first at b=1).
        m_sb = const_pool.tile([128, N_DT * H * D_MODEL], bf16, name="m_sb")
        mq = N_DT * H * D_MODEL // 8

        DEFER_P = 8  # pairs whose dt>=1 matmuls lag: keeps PE fed with light
                     # dt0+Z work at batch transitions while the previous
                     # batch's recip/norm and this batch's W8 pipeline catch up

        ga_v = g_all[:].rearrange("p (dt h bb q) -> p dt h bb q",
                                  dt=N_DT, h=H, bb=B_LOC)

        def epi_norm(b, gps, zps):
            """recip + normalization for batch b.  Norm order dt1..dt3,dt0 so
            the NEXT batch's deferred dt1..3 flush unblocks earliest."""
            zb = zb_pool.tile([128, HQ], f32, name=f"zb_{b}")
            nc.vector.reciprocal_approx_fast(zb[:], zps[:])
            order = [1, 2, 3, 0] if b % 2 == 0 else [0, 1, 2, 3]
            for dt in order:
                nc.vector.tensor_mul(
                    ga_v[:, dt, :, b, :],
                    gps[dt][:].rearrange("p (h q) -> p h q", h=H),
                    zb[:].rearrange("p (h q) -> p h q", h=H),
                )

        def epi_proj(b, interleave_gps_zps=None):
            """output projection for the pair ending at odd batch b."""
            bq = b // 2
            ops = o_pool.tile([128, D_MODEL], f32, tag="o")
            for dt in range(N_DT):
                if interleave_gps_zps is not None:
                    gps, zb = interleave_gps_zps
                    nc.vector.tensor_mul(
                        ga_v[:, dt, :, b, :],
                        gps[dt][:].rearrange("p (h q) -> p h q", h=H),
                        zb[:].rearrange("p (h q) -> p h q", h=H),
                    )
                for h in range(H):
                    col = dt * (H * B_LOC * NQ) + h * (B_LOC * NQ) + bq * 128
                    nc.tensor.matmul(
                        ops[:],
                        g_all[:, col:col + 128],
                        m_sb[:, (dt * H + h) * D_MODEL:
                             (dt * H + h + 1) * D_MODEL],
                        start=(dt == 0 and h == 0),
                        stop=(dt == N_DT - 1 and h == H - 1))
            # b_eff is added on the host
            out_sb = o_sb_pool.tile([128, D_MODEL], f32, tag="osb",
                                    name=f"osb_{bq}")
            nc.vector.tensor_copy(out_sb[:], ops[:])
            nc.sync.dma_start(out.ap()[bq * 128:(bq + 1) * 128, :],
                              out_sb[:])

        pending = None  # (prev_b, prev_gps, prev_zps) epilogue to emit
        pro = prologue(0)
        for b in range(B_LOC):
            a3_t, a12r_t, vs, groups0, w_h0 = pro
            gps = [g_pool.tile([128, HQ], f32, tag=f"g{dt}", name=f"g_{b}_{dt}",
                               bufs=(2 if dt == 0 else 1))
                   for dt in range(N_DT)]
            zps = z_pool.tile([128, HQ], f32, tag="z", name=f"z_{b}", bufs=2)

            # Accumulation openers: Vsum (exact, host) and the NK shift of Z,
            # injected by K=1 matmuls that broadcast a row across PSUM.
            # dt>=1 openers are deferred with their matmuls.
            vsb = vs[:]
            nc.tensor.matmul(gps[0][:], vsb[:, 0:128], ones_row[:],
                             start=True, stop=False)
            nc.tensor.matmul(zps[:], zinit[:], ones_row[:],
                             start=True, stop=False)

            deferred = []
            for half in range(2):
                groups = groups0 if half == 0 else [8, 8]
                f0 = 0
                for gi, FQ in enumerate(groups):
                    if half == 0:
                        w8 = w_h0[gi]
                    else:
                        w8 = emit_group(b, half, gi, FQ, f0, a3_t, a12r_t)
                    w8b = w8[:]
                    for j2 in range(FQ // 2):
                        kp = half * 8 + (f0 + j2 * 2) // 2
                        last = kp == N_KP - 1
                        vt2 = vt_pool.tile([128, 2 * D_IN], f8, tag="vt",
                                           name=f"vt_{b}_{kp}")
                        nc.sync.dma_start(vt2[:], values_p.ap()[b, kp])
                        vtb = vt2[:]
                        rhs = bass.AP(w8b.tensor, w8b.offset + (j2 * 2) * HQ,
                                      [w8b.ap[0], [HQ, 2], [1, HQ]])

                        def g_mm(dt, vtb=vtb, rhs=rhs, last=last):
                            lhsT = bass.AP(vtb.tensor, vtb.offset + dt * 128,
                                           [vtb.ap[0], [D_IN, 2], [1, 128]])
                            nc.tensor.matmul(gps[dt][:], lhsT, rhs,
                                             start=False, stop=last,
                                             perf_mode=DR)

                        g_mm(0)
                        o8 = ones8[:]
                        z_lhsT = bass.AP(o8.tensor, o8.offset,
                                         [o8.ap[0], [128, 2], [1, 128]])
                        nc.tensor.matmul(zps[:], z_lhsT, rhs,
                                         start=False, stop=last, perf_mode=DR)
                        if kp < DEFER_P:
                            deferred.append(g_mm)
                            if kp == DEFER_P - 1:
                                for dt in range(1, N_DT):
                                    nc.tensor.matmul(
                                        gps[dt][:],
                                        vsb[:, dt * 128:(dt + 1) * 128],
                                        ones_row[:], start=True, stop=False)
                                    for mm in deferred:
                                        mm(dt)
                        else:
                            for dt in range(1, N_DT):
                                g_mm(dt)
                        # previous batch's epilogue, staged into this batch's
                        # pair stream: recip+norm early (pair 1), projection
                        # once norms have certainly landed (pair 4)
                        if pending is not None:
                            pb, pgps, pzps = pending
                            if kp == 1:
                                epi_norm(pb, pgps, pzps)
                            elif kp == 4 and pb % 2 == 1:
                                epi_proj(pb)
                            if kp == 4:
                                pending = None
                    f0 += FQ
                win = b * 2 + half
                if win < 4:  # spread m_sb: 2 chunks per half-window of b0/b1
                    for mi in (2 * win, 2 * win + 1):
                        nc.sync.dma_start(m_sb[:, mi * mq:(mi + 1) * mq],
                                          m_all.ap()[:, mi * mq:(mi + 1) * mq])

            if b + 1 < B_LOC:
                pro = prologue(b + 1)
                pending = (b, gps, zps)
            else:
                # last batch: emit its epilogue inline, norm/proj interleaved
                # to keep the tail short
                zb = zb_pool.tile([128, HQ], f32, name=f"zb_{b}")
                nc.vector.reciprocal_approx_fast(zb[:], zps[:])
                epi_proj(b, interleave_gps_zps=(gps, zb))

    nc.compile()
    return nc


def _get_nc():
    if "nc" not in _NC_CACHE:
        _NC_CACHE["nc"] = _build_nc()
    return _NC_CACHE["nc"]


def _host_prep(att12, att3, values, W_v, b_v, W_o, b_o):
    att12 = np.asarray(att12, np.float32)
    att3 = np.asarray(att3, np.float32)
    values = np.asarray(values, np.float32)
    W_v = np.asarray(W_v, np.float32)
    b_v = np.asarray(b_v, np.float32)
    W_o = np.asarray(W_o, np.float32)
    b_o = np.asarray(b_o, np.float32)

    # paired fp8 values: [b, kp, p, i*512+d] = values[b, perm[(2kp+i)*128+p], d]
    values_p = np.ascontiguousarray(
        values[:, _PERM, :].reshape(B, N_KP, 2, 128, D_IN)
        .transpose(0, 1, 3, 2, 4).reshape(B, N_KP, 128, 2 * D_IN)).astype(FP8)
    vsum = values.sum(axis=1).astype(BF16)              # [B, D_IN] exact f32 sum
    att3_t = np.ascontiguousarray(
        att3.transpose(0, 3, 1, 2).reshape(B, NCELL, HQ)).astype(BF16)
    att12_r = np.ascontiguousarray(
        att12.transpose(0, 1, 2, 4, 5, 3).reshape(B, NCELL, F * H)).astype(BF16)
    att12_pair = np.ascontiguousarray(np.broadcast_to(
        att12_r[:, :, :, None], (B, NCELL, F * H, 2)).reshape(
        B, NCELL, F * H * 2))

    # Per-head folded projection M_h = W_o_h @ W_v_h  [D_MODEL, D_IN]
    Wv3 = W_v.reshape(H, D_V, D_IN)
    Wo3 = W_o.reshape(D_MODEL, H, D_V)
    M = np.einsum("dhv,hvi->hdi", Wo3, Wv3)          # [H, DM, DIN]
    Mt = M.transpose(0, 2, 1)                        # [H, DIN, DM]
    m_all = np.ascontiguousarray(
        Mt.reshape(H, N_DT, 128, D_MODEL).transpose(2, 1, 0, 3)
        .reshape(128, N_DT * H * D_MODEL)).astype(BF16)

    b_eff = (b_o + np.einsum("dhv,hv->d", Wo3,
                             b_v.reshape(H, D_V))).astype(np.float32)
    return values_p, vsum, att3_t, att12_pair, m_all, b_eff


def kernel(att12, att3, values, W_v, b_v, W_o, b_o):
    from concourse.bass_utils import run_bass_kernel_spmd

    values_p, vsum, att3_t, att12_pair, m_all, b_eff = _host_prep(
        att12, att3, values, W_v, b_v, W_o, b_o)

    in_maps = []
    for core in range(N_CORES):
        s = slice(core * B_LOC, (core + 1) * B_LOC)
        in_maps.append({
            "values_p": np.ascontiguousarray(values_p[s]),
            "vsum": np.ascontiguousarray(vsum[s]),
            "att3_t": np.ascontiguousarray(att3_t[s]),
            "att12_pair": np.ascontiguousarray(att12_pair[s]),
            "m_all": m_all,
        })

    nc = _get_nc()
    res = run_bass_kernel_spmd(nc, in_maps, core_ids=list(range(N_CORES)))
    out = np.concatenate(
        [res.results[i]["out"].reshape(B_LOC, NQ, D_MODEL)
         for i in range(N_CORES)], axis=0)
    return out.astype(np.float32) + b_eff


# revision 26
# speedup vs baseline: 1.1001x; 1.0921x over previous
"""Trainium2 Bass kernel for nn_BoostEnhancedAttention.

Reference computation:
    v   = (values @ W_v.T + b_v)                      # [B, NK, H*D_V]
    att = softmax(att3 * att12 interleaved, axis=k)   # [B, H, NQ, NK]
    out = (att @ v_per_head) @ W_o.T + b_o            # [B, NQ, D_MODEL]

Restructuring (exact algebra, validated vs reference):
  - Scores factor as s[b,h,q,k] = att3[b,h,q,c(k)] * att12[b,h,...f(k)];
    exp(s) computed by ACT from a DVE-built product grid.
  - Softmax-linearity fold: per-head M_h = W_o[:,h] @ W_v[h,:] applied
    AFTER attention: out[b] = sum_h (att_h @ values[b]) @ M_h.T + b_eff.
  - Shifted-softmax fp8 split: E = 1 + W with W = exp(s) - 1.  Then
        G = sum_k E_k v_k = Vsum + sum_k W_k v_k,   Z = NK + sum_k W_k
    where Vsum = sum_k values[k,:] is computed EXACTLY on the host (f32)
    and injected through a K=1 matmul.  Only the small residual W (rms
    ~0.29x of E) flows through fp8, so both W and values quantize to
    fp8-e4m3 within the error budget.  This enables DoubleRow matmuls:
    each PE instruction contracts TWO 128-key tiles (K=256) at 0.5
    cycles/row - halving both the matmul count and the PE-bound time.
  - Z accumulated on the PE with fp8 DoubleRow ones-matmuls (output
    replicated across partitions), removing the serial DVE add chain.

Sharding: data-parallel over batch, B=32 over 8 cores -> 4 batches/core.
No collectives; outputs concatenated on host.
"""

import numpy as np
import ml_dtypes

B, CH, CW, H, FH, FW = 32, 16, 16, 8, 4, 4
NQ = 64
NCELL = CH * CW          # 256 coarse cells (c)
F = FH * FW              # 16 fine positions per cell
NK = NCELL * F           # 4096
D_IN, D_V, D_MODEL = 512, 64, 512
N_CORES = 8
B_LOC = B // N_CORES     # 4
N_KP = 16                # k-tile PAIRS of 256 keys: kp = half*8 + f//2
N_DT = 4                 # d_in tiles of 128
HQ = H * NQ              # 512

BF16 = ml_dtypes.bfloat16
FP8 = ml_dtypes.float8_e4m3


def _k_perm():
    """perm[k'] -> original k, where k' = (half*16+f)*128 + c_loc.

    Original key order is (ch, fh, cw, fw):  k = ch*256 + fh*64 + cw*4 + fw.
    New order groups a k-tile as (fixed f=(fh,fw), c = half*128 + c_loc).
    """
    perm = np.zeros(NK, np.int64)
    c = np.arange(NCELL)
    ch_i, cw_i = c // CW, c % CW
    for half in range(2):
        for f in range(F):
            kt = half * F + f
            fh, fw = f // FW, f % FW
            cc = half * 128 + np.arange(128)
            perm[kt * 128:(kt + 1) * 128] = (
                ch_i[cc] * (FH * CW * FW) + fh * (CW * FW) + cw_i[cc] * FW + fw
            )
    return perm


_PERM = _k_perm()
_NC_CACHE = {}


def _build_nc():
    from contextlib import ExitStack

    import concourse.bass as bass
    import concourse.tile as tile
    from concourse import bacc, mybir

    f32 = mybir.dt.float32
    bf16 = mybir.dt.bfloat16
    f8 = mybir.dt.float8e4
    DR = mybir.MatmulPerfMode.DoubleRow

    nc = bacc.Bacc("TRN2", target_bir_lowering=False, debug=False,
                   num_devices=N_CORES)

    # values pre-paired on host: [b, kp, p, i*512+d] = values[b, (2kp+i)*128+p, d]
    values_p = nc.dram_tensor("values_p", [B_LOC, N_KP, 128, 2 * D_IN], f8,
                              kind="ExternalInput")
    vsum = nc.dram_tensor("vsum", [B_LOC, D_IN], bf16, kind="ExternalInput")
    att3_t = nc.dram_tensor("att3_t", [B_LOC, NCELL, HQ], bf16,
                            kind="ExternalInput")
    att12_pair = nc.dram_tensor("att12_pair", [B_LOC, NCELL, F * H * 2], bf16,
                                kind="ExternalInput")
    m_all = nc.dram_tensor("m_all", [128, N_DT * H * D_MODEL], bf16,
                           kind="ExternalInput")
    out = nc.dram_tensor("out", [B_LOC * NQ, D_MODEL], f32,
                         kind="ExternalOutput")

    with tile.TileContext(nc) as tc, ExitStack() as ctx:
        const_pool = ctx.enter_context(tc.tile_pool(name="const", bufs=1))
        a3_pool = ctx.enter_context(tc.tile_pool(name="a3", bufs=2))
        a12r_pool = ctx.enter_context(tc.tile_pool(name="a12r", bufs=2))
        vs_pool = ctx.enter_context(tc.tile_pool(name="vs", bufs=2))
        vt_pool = ctx.enter_context(tc.tile_pool(name="vt", bufs=20))
        sc_pool = ctx.enter_context(tc.tile_pool(name="sc", bufs=4))
        et_pool = ctx.enter_context(tc.tile_pool(name="et", bufs=4))
        w8_pool = ctx.enter_context(tc.tile_pool(name="w8", bufs=8))
        zb_pool = ctx.enter_context(tc.tile_pool(name="zb", bufs=2))
        g_pool = ctx.enter_context(tc.tile_pool(name="gps", bufs=1, space="PSUM"))
        z_pool = ctx.enter_context(tc.tile_pool(name="zps", bufs=1, space="PSUM"))
        o_pool = ctx.enter_context(tc.tile_pool(name="ops", bufs=1, space="PSUM"))
        o_sb_pool = ctx.enter_context(tc.tile_pool(name="osb", bufs=2))

        ones_sb = const_pool.tile([128, 128], bf16)
        nc.vector.memset(ones_sb[:], 1.0)
        # fp8 ones for the Z DoubleRow matmuls: [128, 2, 128] -> free 256
        ones8 = const_pool.tile([128, 256], f8)
        nc.vector.memset(ones8[:], 1.0)
        # K=1 helpers: streaming row of ones, and the NK offset for Z
        ones_row = const_pool.tile([1, HQ], bf16)
        nc.vector.memset(ones_row[:], 1.0)
        neg1 = const_pool.tile([128, 1], f32)
        nc.vector.memset(neg1[:], -1.0)
        zinit = const_pool.tile([1, 128], bf16)
        nc.vector.memset(zinit[:], float(NK))
        warm_sb = const_pool.tile([128, D_MODEL], bf16, name="warm_sb")
        nc.vector.memset(warm_sb[:], 1.0)
        warm = o_pool.tile([128, D_MODEL], f32, tag="o", name="warm")
        for wi in range(12):
            nc.tensor.matmul(warm[:], ones_sb[:], warm_sb[:],
                             start=True, stop=True)
        # g_all[d_loc, (dt, h, b, q)] : normalized attention output, bf16
        g_all = const_pool.tile([128, N_DT * H * B_LOC * NQ], bf16)

        Q2 = NQ // 2

        def emit_group(b, half, gi, FQ, f0, a3_t, a12r_t):
            """One score group: broadcast multiply, exp, and W=E-1 in fp8."""
            a3b = a3_t[half][:]
            in0 = bass.AP(a3b.tensor, a3b.offset,
                          [a3b.ap[0], [0, FQ], [NQ, H], [2, Q2], [1, 2]])
            sc = sc_pool.tile([128, 8 * HQ], bf16, tag="sc",
                              name=f"sc_{b}_{half}_{gi}")
            scb = sc[:]
            out_ap = bass.AP(scb.tensor, scb.offset,
                             [scb.ap[0], [HQ, FQ], [NQ, H], [2, Q2], [1, 2]])
            a12b = a12r_t[half][:]
            in1 = bass.AP(a12b.tensor, a12b.offset + f0 * H * 2,
                          [a12b.ap[0], [H * 2, FQ], [2, H], [0, Q2], [1, 2]])
            nc.vector.tensor_mul(out_ap, in0, in1)
            et = et_pool.tile([128, 8 * HQ], bf16, tag="et",
                              name=f"et_{b}_{half}_{gi}")
            nc.scalar.activation(et[:, :FQ * HQ], sc[:, :FQ * HQ],
                                 mybir.ActivationFunctionType.Exp)
            w8 = w8_pool.tile([128, 8 * HQ], f8, tag="w8",
                              name=f"w8_{b}_{half}_{gi}")
            # W = E - 1 (GpSimd measured 23x slower here: fp8 convert is a
            # software path in the Q7 ucode — keep on DVE/ACT).  The last 512
            # columns go to the Scalar engine to balance per-group rates:
            # DVE mul+wsub was ~4.6us/group vs PE 4.3us.
            cut = FQ * HQ - HQ
            nc.vector.tensor_scalar_sub(w8[:, :cut], et[:, :cut], 1.0)
            nc.scalar.activation(w8[:, cut:FQ * HQ], et[:, cut:FQ * HQ],
                                 mybir.ActivationFunctionType.Identity,
                                 bias=neg1[:])
            return w8

        def prologue(b):
            """Input DMAs + first score group for batch b."""
            # half-0 inputs first so group 0's multiply can start after 2 DMAs
            a3_t = [a3_pool.tile([128, HQ], bf16, tag=f"a3_{hf}",
                                 name=f"a3_{b}_{hf}") for hf in range(2)]
            a12r_t = [a12r_pool.tile([128, F * H * 2], bf16, tag=f"a12r_{hf}",
                                     name=f"a12r_{b}_{hf}") for hf in range(2)]
            for hf in range(2):
                nc.sync.dma_start(a3_t[hf][:],
                                  att3_t.ap()[b, hf * 128:(hf + 1) * 128, :])
                nc.sync.dma_start(a12r_t[hf][:],
                                  att12_pair.ap()[b, hf * 128:(hf + 1) * 128, :])
            vs = vs_pool.tile([1, D_IN], bf16, tag="vs", name=f"vs_{b}")
            nc.sync.dma_start(vs[:], vsum.ap()[b:b + 1, :])
            # emit ALL of half 0's score groups plus half 1's small first
            # group here, so the DVE rides far ahead before it hits the
            # previous batch's recip (which blocks DVE until the PE finishes
            # that batch's last Z-matmul)
            groups = [2, 2, 4, 4, 4] if b == 0 else [8, 8]
            w_h0 = []
            f0 = 0
            for gi, FQ in enumerate(groups):
                w_h0.append(emit_group(b, 0, gi, FQ, f0, a3_t, a12r_t))
                f0 += FQ
            w_h1g0 = emit_group(b, 1, 0, GROUPS_H1[0], 0, a3_t, a12r_t)
            return a3_t, a12r_t, vs, groups, w_h0, w_h1g0

        # m_sb DMA is split into 8 chunks spread over early half-windows so it
        # never stalls the critical vt2 prefetch stream (needed first at b=1).
        m_sb = const_pool.tile([128, N_DT * H * D_MODEL], bf16, name="m_sb")
        mq = N_DT * H * D_MODEL // 8

        DEFER_P = 8  # pairs whose dt>=1 matmuls lag: keeps PE fed with light
                     # dt0+Z work at batch transitions while the previous
                     # batch's recip/norm and this batch's W8 pipeline catch up
        GROUPS_H1 = [2, 4, 4, 6]  # small first group: short mul->exp->wsub
                                  # latency right after the recip barrier

        ga_v = g_all[:].rearrange("p (dt h bb q) -> p dt h bb q",
                                  dt=N_DT, h=H, bb=B_LOC)

        def epi_norm(b, gps, zps):
            """recip + normalization for batch b.  Norm order dt1..dt3,dt0 so
            the NEXT batch's deferred dt1..3 flush unblocks earliest."""
            zb = zb_pool.tile([128, HQ], f32, name=f"zb_{b}")
            nc.vector.reciprocal_approx_fast(zb[:], zps[:])
            order = [1, 2, 3, 0] if b % 2 == 0 else [0, 1, 2, 3]
            for dt in order:
                nc.vector.tensor_mul(
                    ga_v[:, dt, :, b, :],
                    gps[dt][:].rearrange("p (h q) -> p h q", h=H),
                    zb[:].rearrange("p (h q) -> p h q", h=H),
                )

        def epi_proj(b, interleave_gps_zps=None):
            """output projection for the pair ending at odd batch b."""
            bq = b // 2
            ops = o_pool.tile([128, D_MODEL], f32, tag="o")
            for dt in range(N_DT):
                if interleave_gps_zps is not None:
                    gps, zb = interleave_gps_zps
                    nc.vector.tensor_mul(
                        ga_v[:, dt, :, b, :],
                        gps[dt][:].rearrange("p (h q) -> p h q", h=H),
                        zb[:].rearrange("p (h q) -> p h q", h=H),
                    )
                for h in range(H):
                    col = dt * (H * B_LOC * NQ) + h * (B_LOC * NQ) + bq * 128
                    nc.tensor.matmul(
                        ops[:],
                        g_all[:, col:col + 128],
                        m_sb[:, (dt * H + h) * D_MODEL:
                             (dt * H + h + 1) * D_MODEL],
                        start=(dt == 0 and h == 0),
                        stop=(dt == N_DT - 1 and h == H - 1))
            # b_eff is added on the host
            out_sb = o_sb_pool.tile([128, D_MODEL], f32, tag="osb",
                                    name=f"osb_{bq}")
            nc.vector.tensor_copy(out_sb[:], ops[:])
            nc.sync.dma_start(out.ap()[bq * 128:(bq + 1) * 128, :],
                              out_sb[:])

        pending = None  # (prev_b, prev_gps, prev_zps) epilogue to emit
        pro = prologue(0)
        for b in range(B_LOC):
            a3_t, a12r_t, vs, groups0, w_h0, w_h1g0 = pro
            gps = [g_pool.tile([128, HQ], f32, tag=f"g{dt}", name=f"g_{b}_{dt}",
                               bufs=(2 if dt == 0 else 1))
                   for dt in range(N_DT)]
            zps = z_pool.tile([128, HQ], f32, tag="z", name=f"z_{b}", bufs=2)

            # Accumulation openers: Vsum (exact, host) and the NK shift of Z,
            # injected by K=1 matmuls that broadcast a row across PSUM.
            # dt>=1 openers are deferred with their matmuls.
            vsb = vs[:]
            nc.tensor.matmul(gps[0][:], vsb[:, 0:128], ones_row[:],
                             start=True, stop=False)
            nc.tensor.matmul(zps[:], zinit[:], ones_row[:],
                             start=True, stop=False)

            deferred = []
            for half in range(2):
                groups = groups0 if half == 0 else GROUPS_H1
                f0 = 0
                for gi, FQ in enumerate(groups):
                    if half == 0:
                        w8 = w_h0[gi]
                    elif gi == 0:
                        w8 = w_h1g0
                    else:
                        w8 = emit_group(b, half, gi, FQ, f0, a3_t, a12r_t)
                    w8b = w8[:]
                    for j2 in range(FQ // 2):
                        kp = half * 8 + (f0 + j2 * 2) // 2
                        last = kp == N_KP - 1
                        vt2 = vt_pool.tile([128, 2 * D_IN], f8, tag="vt",
                                           name=f"vt_{b}_{kp}")
                        nc.sync.dma_start(vt2[:], values_p.ap()[b, kp])
                        vtb = vt2[:]
                        rhs = bass.AP(w8b.tensor, w8b.offset + (j2 * 2) * HQ,
                                      [w8b.ap[0], [HQ, 2], [1, HQ]])

                        def g_mm(dt, vtb=vtb, rhs=rhs, last=last):
                            lhsT = bass.AP(vtb.tensor, vtb.offset + dt * 128,
                                           [vtb.ap[0], [D_IN, 2], [1, 128]])
                            nc.tensor.matmul(gps[dt][:], lhsT, rhs,
                                             start=False, stop=last,
                                             perf_mode=DR)

                        g_mm(0)
                        o8 = ones8[:]
                        z_lhsT = bass.AP(o8.tensor, o8.offset,
                                         [o8.ap[0], [128, 2], [1, 128]])
                        nc.tensor.matmul(zps[:], z_lhsT, rhs,
                                         start=False, stop=last, perf_mode=DR)
                        if kp < DEFER_P:
                            deferred.append(g_mm)
                            if kp == DEFER_P - 1:
                                for dt in range(1, N_DT):
                                    nc.tensor.matmul(
                                        gps[dt][:],
                                        vsb[:, dt * 128:(dt + 1) * 128],
                                        ones_row[:], start=True, stop=False)
                                    for mm in deferred:
                                        mm(dt)
                        else:
                            for dt in range(1, N_DT):
                                g_mm(dt)
                        # previous batch's epilogue, staged into this batch's
                        # pair stream: recip+norm early (pair 1), projection
                        # once norms have certainly landed (pair 4)
                        if pending is not None:
                            pb, pgps, pzps = pending
                            if kp == 1:
                                epi_norm(pb, pgps, pzps)
                            elif kp == 4 and pb % 2 == 1:
                                epi_proj(pb)
                            if kp == 4:
                                pending = None
                    f0 += FQ
                win = b * 2 + half
                if win < 4:  # spread m_sb: 2 chunks per half-window of b0/b1
                    for mi in (2 * win, 2 * win + 1):
                        nc.sync.dma_start(m_sb[:, mi * mq:(mi + 1) * mq],
                                          m_all.ap()[:, mi * mq:(mi + 1) * mq])

            if b + 1 < B_LOC:
                pro = prologue(b + 1)
                pending = (b, gps, zps)
            else:
                # last batch: emit its epilogue inline, norm/proj interleaved
                # to keep the tail short
                zb = zb_pool.tile([128, HQ], f32, name=f"zb_{b}")
                nc.vector.reciprocal_approx_fast(zb[:], zps[:])
                epi_proj(b, interleave_gps_zps=(gps, zb))

    nc.compile()
    return nc


def _get_nc():
    if "nc" not in _NC_CACHE:
        _NC_CACHE["nc"] = _build_nc()
    return _NC_CACHE["nc"]


def _host_prep(att12, att3, values, W_v, b_v, W_o, b_o):
    att12 = np.asarray(att12, np.float32)
    att3 = np.asarray(att3, np.float32)
    values = np.asarray(values, np.float32)
    W_v = np.asarray(W_v, np.float32)
    b_v = np.asarray(b_v, np.float32)
    W_o = np.asarray(W_o, np.float32)
    b_o = np.asarray(b_o, np.float32)

    # paired fp8 values: [b, kp, p, i*512+d] = values[b, perm[(2kp+i)*128+p], d]
    values_p = np.ascontiguousarray(
        values[:, _PERM, :].reshape(B, N_KP, 2, 128, D_IN)
        .transpose(0, 1, 3, 2, 4).reshape(B, N_KP, 128, 2 * D_IN)).astype(FP8)
    vsum = values.sum(axis=1).astype(BF16)              # [B, D_IN] exact f32 sum
    att3_t = np.ascontiguousarray(
        att3.transpose(0, 3, 1, 2).reshape(B, NCELL, HQ)).astype(BF16)
    att12_r = np.ascontiguousarray(
        att12.transpose(0, 1, 2, 4, 5, 3).reshape(B, NCELL, F * H)).astype(BF16)
    att12_pair = np.ascontiguousarray(np.broadcast_to(
        att12_r[:, :, :, None], (B, NCELL, F * H, 2)).reshape(
        B, NCELL, F * H * 2))

    # Per-head folded projection M_h = W_o_h @ W_v_h  [D_MODEL, D_IN]
    Wv3 = W_v.reshape(H, D_V, D_IN)
    Wo3 = W_o.reshape(D_MODEL, H, D_V)
    M = np.einsum("dhv,hvi->hdi", Wo3, Wv3)          # [H, DM, DIN]
    Mt = M.transpose(0, 2, 1)                        # [H, DIN, DM]
    m_all = np.ascontiguousarray(
        Mt.reshape(H, N_DT, 128, D_MODEL).transpose(2, 1, 0, 3)
        .reshape(128, N_DT * H * D_MODEL)).astype(BF16)

    b_eff = (b_o + np.einsum("dhv,hv->d", Wo3,
                             b_v.reshape(H, D_V))).astype(np.float32)
    return values_p, vsum, att3_t, att12_pair, m_all, b_eff


def kernel(att12, att3, values, W_v, b_v, W_o, b_o):
    from concourse.bass_utils import run_bass_kernel_spmd

    values_p, vsum, att3_t, att12_pair, m_all, b_eff = _host_prep(
        att12, att3, values, W_v, b_v, W_o, b_o)

    in_maps = []
    for core in range(N_CORES):
        s = slice(core * B_LOC, (core + 1) * B_LOC)
        in_maps.append({
            "values_p": np.ascontiguousarray(values_p[s]),
            "vsum": np.ascontiguousarray(vsum[s]),
            "att3_t": np.ascontiguousarray(att3_t[s]),
            "att12_pair": np.ascontiguousarray(att12_pair[s]),
            "m_all": m_all,
        })

    nc = _get_nc()
    res = run_bass_kernel_spmd(nc, in_maps, core_ids=list(range(N_CORES)))
    out = np.concatenate(
        [res.results[i]["out"].reshape(B_LOC, NQ, D_MODEL)
         for i in range(N_CORES)], axis=0)
    return out.astype(np.float32) + b_eff
